# revision 52
# baseline (speedup 1.0000x reference)
"""Correlation cost-volume kernel (max_displacement=4) for 8 Trainium2 cores.

Problem: in1, in2: [B=8, C=256, H=128, W=128] f32.
out[b, dy*9+dx, h, w] = sum_c in1[b,c,h,w] * pad(in2)[b, c, h+dy, w+dx]
(pad = 4 zeros on each spatial side), output [8, 81, 128, 128] f32.

Strategy (data-parallel, one batch sample per core):
  The needed outputs are a band-of-band of the cross-gram
  G[(h,w'),(r,w2)] = sum_c in1p[c,h,w'] * in2p[c,r,w2]  (useful iff
  dy = r-h in [0,9) and dx = w2-w' in [0,9)).  Tile it into (TH=8 h-rows
  x TW=8 w2-cols) chunks: one matmul per chunk takes stationary
  in1p[c, 16 w' cols x 8 h rows] (M=128 = the full PE width; in1 is
  host-transposed to [kt,c,hc,w,h] so the block is contiguous - the BIR
  verifier allows only one free dim on the stationary operand) against
  moving in2p[c, 16 r rows, 8 w2 cols] (N=128), covering all 81 (dy,dx)
  pairs of its 8x8 output block with ~3.2x padding waste (vs 15.1x for
  full 136-wide row grams).  C=256 contracts as 2 K=128 matmuls
  accumulated in PSUM (4 w-chunks share one 2KB PSUM bank).

  PSUM chunks are copied (f32->bf16) into a per-2-h-chunk staging tile
  and shipped as dense [128, 2*2176] blocks (8.7KB contiguous per
  partition): shipping only the diagonal band (5.0MB vs 8.9MB) was
  tried via 16-partition-strided DMAs but measured slower - the
  fragmented descriptors drop the DMA engines well below their ~420GB/s
  aggregate.  The host slices the 81 (dy,dx) diagonal planes with one
  vectorized fancy-index per plane (the shear couples output partition
  to free offset, which neither compute engines nor the DMA address
  generator can express).

  Inputs load interior-only: in2's 4 pad rows / cols are memset on-chip
  (they produce real zero outputs); in1's 8-col pads stay uninitialized
  SBUF (they only feed output slots the host discards).  Loads are
  issued in consumption order so the PE starts ~7us in and never
  starves while outputs interleave on the same DMA engines.
"""

import ml_dtypes
import numpy as np

import concourse.bass as bass
import concourse.bacc as bacc
import concourse.mybir as mybir
from concourse.bass_utils import run_bass_kernel_spmd
from concourse.tile import TileContext

B, C, H, W = 8, 256, 128, 128
D = 4
ND = 2 * D + 1  # 9 displacements per axis
KT = C // 128  # 2 contraction tiles
WP = W + 2 * D  # 136 padded in2 width (w2 = w + dx space)
WI = W + 16  # 144: in1 padded by 8 on each side (w' space, offset 8)
TH = 8  # output h rows per chunk
RH = TH + ND - 1  # 16 in2p rows per chunk (r = h + dy)
TW = 8  # w2 cols per chunk
RW = TW + ND - 1  # 16 in1 cols per chunk (w' = w2 - dx)
NHC = H // TH  # 16 h-chunks
# w2 runs over the 128 interior cols only: outputs with w+dx < 4 or >= 132
# hit in2's zero pad by definition, so the host writes those zeros itself.
NWC = W // TW  # 16 w2-chunks
GRP = 4  # w-chunks per PSUM bank (4*128 = 512 f32 = one 2KB bank)
HCG = 1  # h-chunks per staging tile / output DMA
FREE = NWC * RH * TW  # 2176 staging elems per h-chunk
# in2 interior row bands, ends aligned to consumption (hc needs rows < 8hc+12)
B2BANDS = [(0, 20), (20, 44), (44, 68), (68, 92), (92, 116), (116, 128)]

_CACHED_NC = None


def _build_nc():
    bf16 = mybir.dt.bfloat16
    f32 = mybir.dt.float32

    nc = bacc.Bacc()
    in1_t = nc.declare_dram_parameter("in1_t", [KT, 128, NHC, W, TH], bf16, isOutput=False)
    in2_p = nc.declare_dram_parameter("in2_p", [KT, 128, H, W], bf16, isOutput=False)
    # [hc][p=(w'l 16, hl 8)][wc 17, rl 16, w2l 8]
    out_g = nc.declare_dram_parameter("out_g", [NHC, 128, FREE], bf16, isOutput=True)

    wgroups = [list(range(g, min(g + GRP, NWC))) for g in range(0, NWC, GRP)]

    with TileContext(nc) as tc:
        with (
            tc.tile_pool(name="bpool", bufs=1) as bpool,
            tc.tile_pool(name="apool", bufs=1) as apool,
            tc.tile_pool(name="spool", bufs=8) as spool,
            tc.tile_pool(name="psum", bufs=8, space="PSUM") as ppool,
        ):
            b_s = bpool.tile([128, KT, WP, W], bf16)
            a_s = apool.tile([128, KT, NHC, WI, TH], bf16)
            # zero in2p's 4 pad rows top/bottom (pad cols don't exist on-chip)
            for kt in range(KT):
                nc.gpsimd.memset(b_s[:, kt, 0:D, :], 0.0)
                nc.gpsimd.memset(b_s[:, kt, D + H :, :], 0.0)

            # Loads in consumption order; the DGE spreads each instruction's
            # packets across all 16 DMA engines and each engine drains its
            # queue FIFO, so loads complete roughly in issue order at full
            # aggregate bandwidth.
            def load_b(i):
                r0, r1 = B2BANDS[i]
                for kt in range(KT):
                    nc.sync.dma_start(
                        out=b_s[:, kt, D + r0 : D + r1, :],
                        in_=in2_p[kt, :, r0:r1, :],
                    )

            def load_a(h0, h1):  # h-chunks [h0, h1)
                for kt in range(KT):
                    nc.sync.dma_start(
                        out=a_s[:, kt, h0:h1, 8 : 8 + W, :],
                        in_=in1_t[kt, :, h0:h1],
                    )

            # in1 pairs except the last two singles: hc15's compute (the
            # pipeline tail) starts as soon as its own data lands
            for step in ["b0", "a0", "b1", "a1", "a2", "b2", "a3", "b3",
                         "a4", "b4", "a5", "a6", "b5", "a7", "a8"]:
                if step[0] == "b":
                    load_b(int(step[1]))
                else:
                    g = int(step[1])
                    load_a(*((2 * g, 2 * g + 2) if g < 7 else (g + 7, g + 8)))

            for hc in range(NHC):
                h0 = TH * hc
                st = spool.tile([128, FREE], bf16)
                for wg, wcs in enumerate(wgroups):
                    ps = ppool.tile([128, GRP * RH * TW], f32, name=f"ps{wg}", tag="ps")
                    for j, wc in enumerate(wcs):
                        w0 = TW * wc
                        for kt in range(KT):
                            nc.tensor.matmul(
                                ps[:, 128 * j : 128 * j + 128],
                                # w' window for interior w2 chunk: [8wc-4, 8wc+12)
                                a_s[:, kt, hc, w0 + 4 : w0 + 4 + RW, :],
                                b_s[:, kt, h0 : h0 + RH, w0 : w0 + TW],
                                start=(kt == 0),
                                stop=(kt == KT - 1),
                            )
                    nj = len(wcs)
                    c0 = GRP * RH * TW * wg
                    nc.any.tensor_copy(
                        st[:, c0 : c0 + nj * RH * TW],
                        ps[:, : nj * RH * TW],
                    )
                # output DMAs interleave with the input dispatches on the
                # sync queue in program order (A/B-tested faster than
                # dispatching from gpsimd/scalar); two halves per h-chunk so
                # the final transfer after the last copy is half as long.
                half = FREE // 2
                nc.sync.dma_start(out=out_g[hc, :, :half], in_=st[:, :half])
                nc.sync.dma_start(out=out_g[hc, :, half:], in_=st[:, half:])

    nc.compile()
    return nc


def _get_nc():
    global _CACHED_NC
    if _CACHED_NC is None:
        _CACHED_NC = _build_nc()
    return _CACHED_NC


def _make_in_maps(in1: np.ndarray, in2: np.ndarray):
    in_maps = []
    for b in range(B):
        # [kt, c, hc, h_loc, w] -> [kt, c, hc, w, h_loc]
        a = np.ascontiguousarray(
            in1[b]
            .astype(ml_dtypes.bfloat16)
            .reshape(KT, 128, NHC, TH, W)
            .transpose(0, 1, 2, 4, 3)
        )
        p = in2[b].astype(ml_dtypes.bfloat16).reshape(KT, 128, H, W)
        in_maps.append({"in1_t": a, "in2_p": p})
    return in_maps


def _extract_band(g: np.ndarray) -> np.ndarray:
    """[NHC, 128, FREE] dense gram chunks -> [81, H, W] cost volume."""
    rf = np.ascontiguousarray(g).astype(np.float32)
    # [hc, w'l, hl, wc, rl, w2l]
    r6 = rf.reshape(NHC, RW, TH, NWC, RH, TW)
    hl = np.arange(TH)[:, None]  # (8,1)
    w2 = np.arange(TW)[None, :]  # (1,8)
    out = np.zeros((ND * ND, H, W), np.float32)
    for dy in range(ND):
        for dx in range(ND):
            # advanced idx at axes 1,2,4,5 -> (hl,w2l) first, then (hc,wc)
            t = r6[:, w2 + 8 - dx, hl, :, hl + dy, w2]
            wfull = t.transpose(2, 0, 3, 1).reshape(H, NWC * TW)
            # wfull col = w2-4 = w+dx-4; w outside [4-dx, 132-dx) is zero
            lo, hi = max(0, 4 - dx), min(W, 132 - dx)
            out[dy * ND + dx, :, lo:hi] = wfull[:, lo + dx - 4 : hi + dx - 4]
    return out


def kernel(**inputs) -> np.ndarray:
    in1 = np.ascontiguousarray(np.asarray(inputs["in1"], dtype=np.float32))
    in2 = np.ascontiguousarray(np.asarray(inputs["in2"], dtype=np.float32))
    assert in1.shape == (B, C, H, W) and in2.shape == (B, C, H, W)

    nc = _get_nc()
    in_maps = _make_in_maps(in1, in2)
    res = run_bass_kernel_spmd(nc, in_maps, list(range(B)))

    outs = [_extract_band(np.asarray(res.results[b]["out_g"])) for b in range(B)]
    return np.stack(outs).astype(np.float32)
